# revision 1
# baseline (speedup 1.0000x reference)
"""AttentiveFPConv GNN message-passing kernel for 8 Trainium2 NeuronCores.

Reference computation (all fp32):
    alpha = sigmoid(x[col] @ Wa_w + Wa_b)          # per-edge attention
    neigh = x[col] * alpha                          # per-edge message
    aggr  = segment_sum(neigh, row, N)              # per-node aggregation
    out   = tanh(x @ Wn_w + Wn_b + aggr @ Wg_w + Wg_b)

Key algebraic identity: alpha depends only on the source node, so
    h = x * sigmoid(x @ Wa_w + Wa_b)                # per-NODE tensor
    aggr[n] = sum_{e: row[e]=n} h[col[e]]           # gather + segment-sum

Sharding: destination-node sharding. Core k owns nodes [5000k, 5000(k+1))
and ALL edges targeting them (balanced: rows are uniform). No collective
needed: each core computes its own aggr and output slice.

Per-core pipeline:
  Phase 1: h = x*sigmoid(x@Wa+b) for ALL nodes (replicated), h -> HBM bf16.
  Phase 2: dma_gather h[col] in destination-sorted edge order (4 SWDGE
           queues); segment-sum via one-hot matmuls accumulating aggr^T in
           PSUM per 128-node block. One-hot M built by DVE tensor_scalar
           reading a PSUM-resident iota (1-port mode: avoids the exclusive
           DVE<->GpSimd shared-SBUF-port lock that otherwise serializes
           against Q7 gather descriptor generation).
           (dma_gather indices are int16, so edges are split into two
           streams by col < 32768, each gathered against a rebased view.)
  Phase 3: out = tanh(x@Wn + aggr@Wg + ones x bias) -- bias added by a
           rank-1 matmul into the same PSUM accumulation group.
"""

import numpy as np
import ml_dtypes

BF16 = ml_dtypes.bfloat16

# ---------------------------------------------------------------- parameters

class P:
    """Problem/kernel parameters (full-size defaults; shrinkable for tests)."""
    def __init__(self, N=40000, D=128, NCORES=8, HSPLIT=19968,
                 GCHUNK=1024, PH1_CHUNK=2048, NQ=4):
        assert D == 128
        self.N, self.D, self.NCORES = N, D, NCORES
        self.NB = N // NCORES                 # nodes per core
        self.HSPLIT = HSPLIT                  # col split for int16 gather idx
        self.GCHUNK = GCHUNK                  # idxs per dma_gather (HW limit ~1024)
        self.GT = GCHUNK // 128               # gather tiles per chunk
        self.PH1_CHUNK = PH1_CHUNK            # nodes per phase-1 xT chunk
        self.NBLK = (self.NB + 127) // 128    # 128-node blocks per core
        self.NQ = NQ                          # SWDGE queues for dma_gather


# ------------------------------------------------------------ host edge prep

def prep_edges(p: P, row: np.ndarray, col: np.ndarray):
    """Per-core destination-sorted, block-padded edge streams."""
    row = np.asarray(row).astype(np.int64)
    col = np.asarray(col).astype(np.int64)
    cores = []
    for k in range(p.NCORES):
        sel = (row // p.NB) == k
        r = (row[sel] - k * p.NB).astype(np.int32)
        c = col[sel].astype(np.int32)
        order = np.argsort(r, kind="stable")
        r, c = r[order], c[order]
        lo = np.searchsorted(r, np.arange(p.NBLK) * 128)
        hi = np.searchsorted(r, np.minimum(np.arange(1, p.NBLK + 1) * 128, p.NB))
        blocks = []
        for b in range(p.NBLK):
            rb = r[lo[b]:hi[b]] - b * 128
            cb = c[lo[b]:hi[b]]
            mA = cb < p.HSPLIT
            blocks.append(((cb[mA], rb[mA]), (cb[~mA] - p.HSPLIT, rb[~mA])))
        cores.append(blocks)

    nA = np.array([[len(cores[k][b][0][0]) for b in range(p.NBLK)]
                   for k in range(p.NCORES)])
    nB = np.array([[len(cores[k][b][1][0]) for b in range(p.NBLK)]
                   for k in range(p.NCORES)])
    tA = np.maximum(1, -(-nA.max(axis=0) // 128))          # [NBLK]
    tB = np.maximum(1, -(-nB.max(axis=0) // 128))

    LA, LB = int(tA.sum()) * 128, int(tB.sum()) * 128
    LAg = -(-LA // p.GCHUNK) * p.GCHUNK
    LBg = -(-LB // p.GCHUNK) * p.GCHUNK

    per_core = []
    for k in range(p.NCORES):
        idxA = np.zeros(LAg, np.int16); lrA = np.full(LA, -1.0, np.float32)
        idxB = np.zeros(LBg, np.int16); lrB = np.full(LB, -1.0, np.float32)
        oA = oB = 0
        for b in range(p.NBLK):
            (cA, rA), (cB, rB) = cores[k][b]
            idxA[oA:oA + len(cA)] = cA; lrA[oA:oA + len(rA)] = rA
            oA += int(tA[b]) * 128
            idxB[oB:oB + len(cB)] = cB; lrB[oB:oB + len(rB)] = rB
            oB += int(tB[b]) * 128
        per_core.append({
            "idxA": np.tile(idxA.reshape(-1, 16).T, (8, 1)),   # [128, LAg/16]
            "idxB": np.tile(idxB.reshape(-1, 16).T, (8, 1)),
            "lrA": lrA.reshape(-1, 128).T.copy(),              # [128, LA/128]
            "lrB": lrB.reshape(-1, 128).T.copy(),
        })
    return tA, tB, LA, LB, LAg, LBg, per_core


# ------------------------------------------------------------- device kernel

def build(p: P, tA, tB, LA, LB, LAg, LBg):
    from concourse import bacc, mybir, tile

    f32, bf16, i16 = mybir.dt.float32, mybir.dt.bfloat16, mybir.dt.int16
    AF = mybir.ActivationFunctionType
    nc = bacc.Bacc("TRN2", target_bir_lowering=False, debug=False,
                   num_devices=p.NCORES, num_swdge_queues=p.NQ)

    N, D, NB, NBLK = p.N, p.D, p.NB, p.NBLK
    H = p.HSPLIT                    # h1 rows; h2 rows = N - H
    N2 = N - H
    assert H % 128 == 0

    xT_d   = nc.dram_tensor("xT", [D, N], bf16, kind="ExternalInput")
    xTo_d  = nc.dram_tensor("xT_own", [D, NB], bf16, kind="ExternalInput")
    WaW_d  = nc.dram_tensor("WaW", [D, D], bf16, kind="ExternalInput")
    WaB_d  = nc.dram_tensor("WaB", [D, 1], f32, kind="ExternalInput")
    WnW_d  = nc.dram_tensor("WnW", [D, D], bf16, kind="ExternalInput")
    WgW_d  = nc.dram_tensor("WgW", [D, D], bf16, kind="ExternalInput")
    bias_d = nc.dram_tensor("biasR", [1, D], bf16, kind="ExternalInput")
    ones_d = nc.dram_tensor("onesR", [1, D], bf16, kind="ExternalInput")
    ident_d= nc.dram_tensor("ident", [D, D], bf16, kind="ExternalInput")
    idxA_d = nc.dram_tensor("idxA", [128, LAg // 16], i16, kind="ExternalInput")
    idxB_d = nc.dram_tensor("idxB", [128, LBg // 16], i16, kind="ExternalInput")
    MA_d   = nc.dram_tensor("MA", [128, LA // 128, D], bf16, kind="ExternalInput")
    MB_d   = nc.dram_tensor("MB", [128, LB // 128, D], bf16, kind="ExternalInput")
    out_d  = nc.dram_tensor("out", [NB, D], f32, kind="ExternalOutput")
    h1_d   = nc.dram_tensor("h1", [H, D], bf16, kind="Internal")
    h2_d   = nc.dram_tensor("h2", [N2, D], bf16, kind="Internal")

    PIECE = 9984                   # nodes per hT staging piece (78 blocks)

    with tile.TileContext(nc) as tc:
        with (
            tc.tile_pool(name="const", bufs=1) as cpool,
            tc.tile_pool(name="xchunk", bufs=4) as xpool,
            tc.tile_pool(name="hT", bufs=2) as htpool,
            tc.tile_pool(name="hstage", bufs=1) as hspool,
            tc.tile_pool(name="pg", bufs=2, space="PSUM") as pg_pool,
            tc.tile_pool(name="pt", bufs=1, space="PSUM") as pt_pool,
            tc.tile_pool(name="pa", bufs=2, space="PSUM") as pa_pool,
            tc.tile_pool(name="po", bufs=2, space="PSUM") as po_pool,
            tc.tile_pool(name="sA", bufs=13) as gApool,
            tc.tile_pool(name="sB", bufs=13) as gBpool,
            tc.tile_pool(name="m", bufs=3) as mpool,
            tc.tile_pool(name="aggA", bufs=(NBLK + 3) // 4) as aggApool,
            tc.tile_pool(name="aggB", bufs=3) as aggBpool,
            tc.tile_pool(name="ph1w", bufs=4) as w1pool,
            tc.tile_pool(name="ostage", bufs=2) as ospool,
        ):
            # ---- constants into SBUF
            WaW = cpool.tile([D, D], bf16); nc.sync.dma_start(out=WaW[:], in_=WaW_d[:])
            WaB = cpool.tile([D, 1], f32); nc.sync.dma_start(out=WaB[:], in_=WaB_d[:])
            WnW = cpool.tile([D, D], bf16); nc.sync.dma_start(out=WnW[:], in_=WnW_d[:])
            WgW = cpool.tile([D, D], bf16); nc.sync.dma_start(out=WgW[:], in_=WgW_d[:])
            biasR = cpool.tile([1, D], bf16); nc.sync.dma_start(out=biasR[:], in_=bias_d[:])
            onesR = cpool.tile([1, D], bf16); nc.sync.dma_start(out=onesR[:], in_=ones_d[:])
            ident = cpool.tile([D, D], bf16); nc.sync.dma_start(out=ident[:], in_=ident_d[:])
            xT_own = cpool.tile([D, NB], bf16); nc.sync.dma_start(out=xT_own[:], in_=xTo_d[:])
            idxA_sb = cpool.tile([128, LAg // 16], i16)
            nc.sync.dma_start(out=idxA_sb[:], in_=idxA_d[:])
            idxB_sb = cpool.tile([128, LBg // 16], i16)
            nc.sync.dma_start(out=idxB_sb[:], in_=idxB_d[:])


            # ---- phase 1: h = x * sigmoid(x@Wa + b); hT pieces -> xbar -> HBM
            def ph1_compute(hTp, base, cn):
                """Compute hT for nodes [base, base+cn) into hTp[:, :cn]."""
                off = 0
                while off < cn:
                    w = min(2048, cn - off)
                    xc = xpool.tile([D, 2048], bf16, tag="xc")
                    nc.sync.dma_start(out=xc[:, :w], in_=xT_d[:, base + off:base + off + w])
                    g0 = 0
                    while g0 < w:
                        gw = min(512, w - g0)
                        pg = pg_pool.tile([D, 512], f32, tag="pg")
                        nc.tensor.matmul(pg[:, :gw], lhsT=WaW[:],
                                         rhs=xc[:, g0:g0 + gw], start=True, stop=True)
                        sT = w1pool.tile([D, 512], bf16, tag="sT")
                        nc.scalar.activation(sT[:, :gw], pg[:, :gw], AF.Sigmoid,
                                             bias=WaB[:, 0:1])
                        nc.vector.tensor_tensor(out=hTp[:, off + g0:off + g0 + gw],
                                                in0=xc[:, g0:g0 + gw],
                                                in1=sT[:, :gw], op=mybir.AluOpType.mult)
                        g0 += gw
                    off += w

            def ph1_flush(hTp, h_t, base_in_h, cn, use_xbar):
                """Transpose hTp[:, :cn] and DMA to h_t rows [base_in_h, +cn)."""
                nfull = cn // 128
                rem = cn - nfull * 128
                if nfull and use_xbar:
                    hst = hspool.tile([128, PIECE // 128, 128], bf16, tag="hst")
                    nc.sync.dma_start_transpose(hst[:, :nfull, :], hTp[:, :nfull * 128])
                    nc.sync.dma_start(
                        out=h_t[base_in_h:base_in_h + nfull * 128, :].rearrange(
                            "(t p) d -> p t d", p=128),
                        in_=hst[:, :nfull, :])
                elif nfull:
                    # PE-transpose path (avoids xbar mode-serialization of the
                    # DMA stream while gathers/M-streams are in flight)
                    hst = hspool.tile([128, PIECE // 128, 128], bf16, tag="hst")
                    t0 = 0
                    while t0 < nfull:
                        tn = min(4, nfull - t0)
                        pt = pt_pool.tile([128, 512], bf16, tag="pt")
                        for q in range(tn):
                            nc.tensor.transpose(
                                pt[:, q * 128:(q + 1) * 128],
                                hTp[:, (t0 + q) * 128:(t0 + q + 1) * 128], ident[:])
                        nc.vector.tensor_copy(
                            out=hst[:, t0:t0 + tn, :].rearrange("p t d -> p (t d)"),
                            in_=pt[:, :tn * 128])
                        t0 += tn
                    nc.sync.dma_start(
                        out=h_t[base_in_h:base_in_h + nfull * 128, :].rearrange(
                            "(t p) d -> p t d", p=128),
                        in_=hst[:, :nfull, :])
                if rem:
                    pt = pt_pool.tile([128, 512], bf16, tag="pt")
                    nc.tensor.transpose(pt[:rem, :128], hTp[:, nfull * 128:nfull * 128 + rem],
                                        ident[:])
                    tl = w1pool.tile([128, 128], bf16, tag="tail")
                    nc.vector.tensor_copy(out=tl[:rem, :], in_=pt[:rem, :128])
                    nc.sync.dma_start(
                        out=h_t[base_in_h + nfull * 128:base_in_h + cn, :],
                        in_=tl[:rem, :])

            # h1: nodes [0, H)
            base = 0
            while base < H:
                cn = min(PIECE, H - base)
                hTp = htpool.tile([D, PIECE], bf16, tag="hT")
                ph1_compute(hTp, base, cn)
                ph1_flush(hTp, h1_d, base, cn, use_xbar=True)
                base += cn
            # h2: nodes [H, N)  (PE transposes: keep the DMA stream xbar-free
            # while pass-A gathers and M-streams are running)
            while base < N:
                cn = min(PIECE, N - base)
                hTp = htpool.tile([D, PIECE], bf16, tag="hT")
                ph1_compute(hTp, base, cn)
                ph1_flush(hTp, h2_d, base - H, cn, use_xbar=False)
                base += cn

            # ---- phase 2: two passes (A from h1, B from h2), one-hot scatter
            nq_counter = [0]
            gA_tiles = [None] * (LAg // p.GCHUNK)
            gB_tiles = [None] * (LBg // p.GCHUNK)
            MGT = 16
            mA_tiles = [None] * (-(-(LA // 128) // MGT))
            mB_tiles = [None] * (-(-(LB // 128) // MGT))

            def ensure_chunk(tiles, which, ci):
                if tiles[ci] is not None:
                    return
                g = (gApool if which == "A" else gBpool).tile(
                    [128, p.GT, D], bf16, tag="g" + which)
                idx_sb = idxA_sb if which == "A" else idxB_sb
                src = h1_d[:, :] if which == "A" else h2_d[:, :]
                c0 = ci * (p.GCHUNK // 16)
                nc.gpsimd.dma_gather(
                    out_ap=g[:], in_ap=src, idxs_ap=idx_sb[:, c0:c0 + p.GCHUNK // 16],
                    num_idxs=p.GCHUNK, num_idxs_reg=p.GCHUNK, elem_size=D,
                    queue_num=nq_counter[0] % p.NQ)
                nq_counter[0] += 1
                tiles[ci] = g

            def ensure_mchunk(tiles, which, ci):
                if tiles[ci] is not None:
                    return
                md = MA_d if which == "A" else MB_d
                nt = md.shape[1]
                t0 = ci * MGT
                tn = min(MGT, nt - t0)
                mt = mpool.tile([128, MGT, D], bf16, tag="m" + which)
                nc.sync.dma_start(out=mt[:, :tn, :], in_=md[:, t0:t0 + tn, :])
                tiles[ci] = mt

            posA = np.concatenate([[0], np.cumsum(tA)]).astype(int)
            posB = np.concatenate([[0], np.cumsum(tB)]).astype(int)
            NG = (NBLK + 3) // 4

            def scatter_pass(tcnts, pos, gtiles, mtiles, which, aggpool):
                """One-hot matmul scatter for one stream; returns agg tiles."""
                aggs = []
                b0 = 0
                while b0 < NBLK:
                    gn = min(4, NBLK - b0)
                    pa = pa_pool.tile([D, 512], f32, tag="pa")
                    for q in range(gn):
                        b = b0 + q
                        tcnt = int(tcnts[b])
                        for j in range(tcnt):
                            g = pos[b] + j
                            ensure_chunk(gtiles, which, g // p.GT)
                            ensure_mchunk(mtiles, which, g // MGT)
                            neigh = gtiles[g // p.GT][:, g % p.GT, :]
                            M = mtiles[g // MGT][:, g % MGT, :]
                            nc.tensor.matmul(pa[:, q * 128:(q + 1) * 128],
                                             lhsT=neigh, rhs=M,
                                             start=(j == 0), stop=(j == tcnt - 1))
                    agg = aggpool.tile([D, 512], bf16, tag="agg" + which)
                    nc.vector.tensor_copy(out=agg[:], in_=pa[:])
                    aggs.append(agg)
                    b0 += gn
                return aggs

            aggsA = scatter_pass(tA, posA, gA_tiles, mA_tiles, "A", aggApool)

            # ---- pass B + phase 3 fused per 4-block group
            OCH = 8
            ost = None
            ost_base = 0
            ost_n = 0
            b0 = 0
            while b0 < NBLK:
                gn = min(4, NBLK - b0)
                gi = b0 // 4
                pa = pa_pool.tile([D, 512], f32, tag="pa")
                for q in range(gn):
                    b = b0 + q
                    tcnt = int(tB[b])
                    for j in range(tcnt):
                        g = posB[b] + j
                        ensure_chunk(gB_tiles, "B", g // p.GT)
                        ensure_mchunk(mB_tiles, "B", g // MGT)
                        neigh = gB_tiles[g // p.GT][:, g % p.GT, :]
                        M = mB_tiles[g // MGT][:, g % MGT, :]
                        nc.tensor.matmul(pa[:, q * 128:(q + 1) * 128],
                                         lhsT=neigh, rhs=M,
                                         start=(j == 0), stop=(j == tcnt - 1))
                aggB = aggBpool.tile([D, 512], bf16, tag="aggB")
                nc.vector.tensor_copy(out=aggB[:], in_=pa[:])

                po = po_pool.tile([128, 512], f32, tag="po")
                for q in range(gn):
                    b = b0 + q
                    nb = min(128, NB - b * 128)
                    sl = slice(q * 128, q * 128 + D)
                    nc.tensor.matmul(po[:nb, sl],
                                     lhsT=xT_own[:, b * 128:b * 128 + nb],
                                     rhs=WnW[:], start=True, stop=False)
                    nc.tensor.matmul(po[:nb, sl],
                                     lhsT=aggsA[gi][:, q * 128:q * 128 + nb],
                                     rhs=WgW[:], start=False, stop=False)
                    nc.tensor.matmul(po[:nb, sl],
                                     lhsT=aggB[:, q * 128:q * 128 + nb],
                                     rhs=WgW[:], start=False, stop=False)
                    nc.tensor.matmul(po[:nb, sl], lhsT=onesR[:1, :nb],
                                     rhs=biasR[:1, :], start=False, stop=True)
                for q in range(gn):
                    b = b0 + q
                    nb = min(128, NB - b * 128)
                    if ost is None:
                        ost = ospool.tile([128, OCH * D], f32, tag="ost")
                        ost_base = b
                        ost_n = 0
                    nc.scalar.activation(ost[:nb, ost_n * D:(ost_n + 1) * D],
                                         po[:nb, q * 128:q * 128 + D], AF.Tanh)
                    ost_n += 1
                    if (ost_n == OCH) or (b == NBLK - 1):
                        rows0 = ost_base * 128
                        nfull_o = ost_n if nb == 128 else ost_n - 1
                        if nfull_o:
                            nc.sync.dma_start(
                                out=out_d[rows0:rows0 + nfull_o * 128, :].rearrange(
                                    "(t p) d -> p t d", p=128),
                                in_=ost[:, :nfull_o * D].rearrange(
                                    "p (t d) -> p t d", d=D))
                        if nb != 128:
                            nc.sync.dma_start(
                                out=out_d[rows0 + nfull_o * 128:
                                          rows0 + nfull_o * 128 + nb, :],
                                in_=ost[:nb, nfull_o * D:nfull_o * D + D])
                        ost = None
                b0 += gn

    nc.compile()
    return nc


# ---------------------------------------------------------------- host entry

def _host_prep(p: P, x, edge_index, Wn_w, Wn_b, Wg_w, Wg_b, Wa_w, Wa_b):
    x = np.asarray(x, np.float32)
    xT = np.ascontiguousarray(x.T).astype(BF16)
    tA, tB, LA, LB, LAg, LBg, per_core = prep_edges(
        p, np.asarray(edge_index)[0], np.asarray(edge_index)[1])

    shared = {
        "xT": xT,
        "WaW": np.asarray(Wa_w, np.float32).astype(BF16),
        "WaB": np.asarray(Wa_b, np.float32).reshape(p.D, 1),
        "WnW": np.asarray(Wn_w, np.float32).astype(BF16),
        "WgW": np.asarray(Wg_w, np.float32).astype(BF16),
        "biasR": (np.asarray(Wn_b, np.float32)
                  + np.asarray(Wg_b, np.float32)).reshape(1, p.D).astype(BF16),
        "onesR": np.ones((1, p.D), BF16),
        "ident": np.eye(p.D, dtype=np.float32).astype(BF16),
    }
    in_maps = []
    for k in range(p.NCORES):
        m = dict(shared)
        m["xT_own"] = np.ascontiguousarray(xT[:, k * p.NB:(k + 1) * p.NB])
        pc = per_core[k]
        m["idxA"], m["idxB"] = pc["idxA"], pc["idxB"]
        jj = np.arange(p.D, dtype=np.float32)[None, None, :]
        m["MA"] = (pc["lrA"][:, :, None] == jj).astype(BF16)
        m["MB"] = (pc["lrB"][:, :, None] == jj).astype(BF16)
        in_maps.append(m)
    return tA, tB, LA, LB, LAg, LBg, in_maps


TRACE = False      # set True (e.g. from test.py) to capture an NTFF profile
LAST = None        # last BassKernelResults, for profiling/inspection


def kernel(**inputs) -> np.ndarray:
    global LAST
    from concourse import bass_utils
    bass_utils.upload_artifacts = lambda tmpdir: "local://" + tmpdir

    p = P()
    tA, tB, LA, LB, LAg, LBg, in_maps = _host_prep(p, **inputs)
    nc = build(p, tA, tB, LA, LB, LAg, LBg)
    kw = dict(trace=True, trace_cores=list(range(p.NCORES))) if TRACE else {}
    res = bass_utils.run_bass_kernel_spmd(
        nc, in_maps, core_ids=list(range(p.NCORES)), **kw)
    LAST = res
    out = np.concatenate([res.results[k]["out"] for k in range(p.NCORES)], axis=0)
    return out.astype(np.float32)



# revision 8
# speedup vs baseline: 1.2226x; 1.2226x over previous
"""AttentiveFPConv GNN message-passing kernel for 8 Trainium2 NeuronCores.

Reference computation (all fp32):
    alpha = sigmoid(x[col] @ Wa_w + Wa_b)          # per-edge attention
    neigh = x[col] * alpha                          # per-edge message
    aggr  = segment_sum(neigh, row, N)              # per-node aggregation
    out   = tanh(x @ Wn_w + Wn_b + aggr @ Wg_w + Wg_b)

Key algebraic identity: alpha depends only on the source node, so
    h = x * sigmoid(x @ Wa_w + Wa_b)                # per-NODE tensor
    aggr[n] = sum_{e: row[e]=n} h[col[e]]           # gather + segment-sum

Sharding: destination-node sharding. Core k owns nodes [5000k, 5000(k+1))
and ALL edges targeting them (balanced: rows are uniform). No collective
needed: each core computes its own aggr and output slice.

Per-core pipeline:
  Phase 1: h = x*sigmoid(x@Wa+b) for ALL nodes (replicated). h stored in
           HBM partition-major as [128, T, D] so the store DMA has large
           contiguous per-partition runs (full DMA rate); the gather
           addresses node c at row (c%128)*T + c//128.
  Phase 2: dma_gather h[col] in destination-sorted edge order (4 SWDGE
           queues, 2048-idx chunks); segment-sum via one-hot matmuls
           accumulating aggr^T in PSUM per 128-node block. One-hot M is
           built ON-CHIP: DVE tensor_tensor is_equal of a PSUM-resident
           iota against the per-edge local-row stream (bcast AP), 4 tiles
           per op. (dma_gather indices are int16, so edges are split into
           two streams by col < HSPLIT, each gathered against its own
           rebased h tensor.)
  Phase 3: out = tanh(x@Wn + aggr@Wg + ones x bias) -- bias added by a
           rank-1 matmul into the same PSUM accumulation group.
"""

import numpy as np
import ml_dtypes

BF16 = ml_dtypes.bfloat16

# ---------------------------------------------------------------- parameters

class P:
    """Problem/kernel parameters (full-size defaults; shrinkable for tests)."""
    def __init__(self, N=40000, D=128, NCORES=8, HSPLIT=19968,
                 GCHUNK=1024, PH1_CHUNK=2048, NQ=4, SCRATCH=16384,
                 PIECE=4992, BATCH_M=True):
        assert D == 128
        self.N, self.D, self.NCORES = N, D, NCORES
        self.NB = N // NCORES                 # nodes per core
        self.HSPLIT = HSPLIT                  # col split for int16 gather idx
        self.GCHUNK = GCHUNK                  # idxs per dma_gather call
        self.GT = GCHUNK // 128               # gather tiles per chunk
        self.PH1_CHUNK = PH1_CHUNK            # nodes per phase-1 xT chunk
        self.NBLK = (self.NB + 127) // 128    # 128-node blocks per core
        self.NQ = NQ                          # SWDGE queues for dma_gather
        self.SCRATCH = SCRATCH                # SWDGE desc ring bytes/partition
        self.PIECE = PIECE                    # nodes per phase-1 staging piece
        self.BATCH_M = BATCH_M                # build one-hot 4 tiles per DVE op
        self.T1 = HSPLIT // 128               # h1 tiles (partition-major)
        self.T2 = (N - HSPLIT + 127) // 128   # h2 tiles


# ------------------------------------------------------------ host edge prep

def prep_edges(p: P, row: np.ndarray, col: np.ndarray):
    """Per-core destination-sorted, block-padded edge streams."""
    row = np.asarray(row).astype(np.int64)
    col = np.asarray(col).astype(np.int64)
    cores = []
    for k in range(p.NCORES):
        sel = (row // p.NB) == k
        r = (row[sel] - k * p.NB).astype(np.int32)
        c = col[sel].astype(np.int32)
        order = np.argsort(r, kind="stable")
        r, c = r[order], c[order]
        lo = np.searchsorted(r, np.arange(p.NBLK) * 128)
        hi = np.searchsorted(r, np.minimum(np.arange(1, p.NBLK + 1) * 128, p.NB))
        blocks = []
        for b in range(p.NBLK):
            rb = r[lo[b]:hi[b]] - b * 128
            cb = c[lo[b]:hi[b]]
            mA = cb < p.HSPLIT
            cA = cb[mA]
            c2 = cb[~mA] - p.HSPLIT
            # partition-major h layout: node c -> row (c%128)*T + c//128
            gA = (cA % 128) * p.T1 + cA // 128
            gB = (c2 % 128) * p.T2 + c2 // 128
            blocks.append(((gA, rb[mA]), (gB, rb[~mA])))
        cores.append(blocks)

    nA = np.array([[len(cores[k][b][0][0]) for b in range(p.NBLK)]
                   for k in range(p.NCORES)])
    nB = np.array([[len(cores[k][b][1][0]) for b in range(p.NBLK)]
                   for k in range(p.NCORES)])
    tA = np.maximum(1, -(-nA.max(axis=0) // 128))          # [NBLK]
    tB = np.maximum(1, -(-nB.max(axis=0) // 128))

    LA, LB = int(tA.sum()) * 128, int(tB.sum()) * 128
    LAg = -(-LA // p.GCHUNK) * p.GCHUNK
    LBg = -(-LB // p.GCHUNK) * p.GCHUNK

    per_core = []
    for k in range(p.NCORES):
        idxA = np.zeros(LAg, np.int16); lrA = np.full(LA, -1.0, np.float32)
        idxB = np.zeros(LBg, np.int16); lrB = np.full(LB, -1.0, np.float32)
        oA = oB = 0
        for b in range(p.NBLK):
            (cA, rA), (cB, rB) = cores[k][b]
            idxA[oA:oA + len(cA)] = cA; lrA[oA:oA + len(rA)] = rA
            oA += int(tA[b]) * 128
            idxB[oB:oB + len(cB)] = cB; lrB[oB:oB + len(rB)] = rB
            oB += int(tB[b]) * 128
        per_core.append({
            "idxA": np.tile(idxA.reshape(-1, 16).T, (8, 1)),   # [128, LAg/16]
            "idxB": np.tile(idxB.reshape(-1, 16).T, (8, 1)),
            "lrA": lrA.reshape(-1, 128).T.copy(),              # [128, LA/128]
            "lrB": lrB.reshape(-1, 128).T.copy(),
        })
    return tA, tB, LA, LB, LAg, LBg, per_core


# ------------------------------------------------------------- device kernel

def build(p: P, tA, tB, LA, LB, LAg, LBg):
    from concourse import bacc, mybir, tile

    f32, bf16, i16 = mybir.dt.float32, mybir.dt.bfloat16, mybir.dt.int16
    AF = mybir.ActivationFunctionType
    nc = bacc.Bacc("TRN2", target_bir_lowering=False, debug=False,
                   num_devices=p.NCORES, num_swdge_queues=p.NQ,
                   dynamic_dma_scratch_size=p.SCRATCH)

    N, D, NB, NBLK = p.N, p.D, p.NB, p.NBLK
    H = p.HSPLIT                    # h1 nodes; h2 nodes = N - H
    T1, T2 = p.T1, p.T2
    PIECE = p.PIECE
    assert H % 128 == 0 and H % PIECE == 0
    SA, SB = LA // 128, LB // 128   # one-hot tiles per stream

    xT_d   = nc.dram_tensor("xT", [D, N], bf16, kind="ExternalInput")
    xTo_d  = nc.dram_tensor("xT_own", [D, NB], bf16, kind="ExternalInput")
    WaW_d  = nc.dram_tensor("WaW", [D, D], bf16, kind="ExternalInput")
    WaB_d  = nc.dram_tensor("WaB", [D, 1], f32, kind="ExternalInput")
    WnW_d  = nc.dram_tensor("WnW", [D, D], bf16, kind="ExternalInput")
    WgW_d  = nc.dram_tensor("WgW", [D, D], bf16, kind="ExternalInput")
    bias_d = nc.dram_tensor("biasR", [1, D], bf16, kind="ExternalInput")
    ones_d = nc.dram_tensor("onesR", [1, D], bf16, kind="ExternalInput")
    ident_d= nc.dram_tensor("ident", [D, D], bf16, kind="ExternalInput")
    iota_d = nc.dram_tensor("iota4", [128, 512], bf16, kind="ExternalInput")
    idxA_d = nc.dram_tensor("idxA", [128, LAg // 16], i16, kind="ExternalInput")
    idxB_d = nc.dram_tensor("idxB", [128, LBg // 16], i16, kind="ExternalInput")
    lrA_d  = nc.dram_tensor("lrA", [128, SA], f32, kind="ExternalInput")
    lrB_d  = nc.dram_tensor("lrB", [128, SB], f32, kind="ExternalInput")
    out_d  = nc.dram_tensor("out", [NB, D], f32, kind="ExternalOutput")
    h1_d   = nc.dram_tensor("h1", [128, T1, D], bf16, kind="Internal")
    h2_d   = nc.dram_tensor("h2", [128, T2, D], bf16, kind="Internal")
    h1_rows = h1_d[:, :, :].rearrange("p t d -> (p t) d")
    h2_rows = h2_d[:, :, :].rearrange("p t d -> (p t) d")

    with tile.TileContext(nc) as tc:
        with (
            tc.tile_pool(name="const", bufs=1) as cpool,
            tc.tile_pool(name="xchunk", bufs=3) as xpool,
            tc.tile_pool(name="hT", bufs=2) as htpool,
            tc.tile_pool(name="hstage", bufs=1) as hspool,
            tc.tile_pool(name="pg", bufs=2, space="PSUM") as pg_pool,
            tc.tile_pool(name="pt", bufs=1, space="PSUM") as pt_pool,
            tc.tile_pool(name="pi", bufs=1, space="PSUM") as pi_pool,
            tc.tile_pool(name="pa", bufs=2, space="PSUM") as pa_pool,
            tc.tile_pool(name="po", bufs=2, space="PSUM") as po_pool,
            tc.tile_pool(name="sA", bufs=6) as gApool,
            tc.tile_pool(name="sB", bufs=6) as gBpool,
            tc.tile_pool(name="m", bufs=6) as mpool,
            tc.tile_pool(name="aggA", bufs=(NBLK + 3) // 4) as aggApool,
            tc.tile_pool(name="aggB", bufs=3) as aggBpool,
            tc.tile_pool(name="ph1w", bufs=4) as w1pool,
            tc.tile_pool(name="ostage", bufs=2) as ospool,
        ):
            # ---- constants into SBUF
            WaW = cpool.tile([D, D], bf16); nc.sync.dma_start(out=WaW[:], in_=WaW_d[:])
            WaB = cpool.tile([D, 1], f32); nc.sync.dma_start(out=WaB[:], in_=WaB_d[:])
            WnW = cpool.tile([D, D], bf16); nc.sync.dma_start(out=WnW[:], in_=WnW_d[:])
            WgW = cpool.tile([D, D], bf16); nc.sync.dma_start(out=WgW[:], in_=WgW_d[:])
            biasR = cpool.tile([1, D], bf16); nc.sync.dma_start(out=biasR[:], in_=bias_d[:])
            onesR = cpool.tile([1, D], bf16); nc.sync.dma_start(out=onesR[:], in_=ones_d[:])
            ident = cpool.tile([D, D], bf16); nc.sync.dma_start(out=ident[:], in_=ident_d[:])
            iota4 = cpool.tile([128, 512], bf16); nc.sync.dma_start(out=iota4[:], in_=iota_d[:])
            xT_own = cpool.tile([D, NB], bf16); nc.sync.dma_start(out=xT_own[:], in_=xTo_d[:])
            lrA_sb = cpool.tile([128, SA], f32); nc.sync.dma_start(out=lrA_sb[:], in_=lrA_d[:])
            lrB_sb = cpool.tile([128, SB], f32); nc.sync.dma_start(out=lrB_sb[:], in_=lrB_d[:])
            idxA_sb = cpool.tile([128, LAg // 16], i16)
            nc.sync.dma_start(out=idxA_sb[:], in_=idxA_d[:])
            idxB_sb = cpool.tile([128, LBg // 16], i16)
            nc.sync.dma_start(out=idxB_sb[:], in_=idxB_d[:])

            # PSUM-resident iota (values 0..127 x4), built once via matmul
            piota = pi_pool.tile([128, 512], f32)
            nc.tensor.matmul(piota[:], lhsT=ident[:], rhs=iota4[:],
                             start=True, stop=True)

            # ---- phase 1: h = x * sigmoid(x@Wa + b); hT pieces -> h[128,T,D]
            def ph1_compute(hTp, base, cn):
                """Compute hT for nodes [base, base+cn) into hTp[:, :cn]."""
                off = 0
                while off < cn:
                    w = min(p.PH1_CHUNK, cn - off)
                    xc = xpool.tile([D, p.PH1_CHUNK], bf16, tag="xc")
                    nc.sync.dma_start(out=xc[:, :w], in_=xT_d[:, base + off:base + off + w])
                    g0 = 0
                    while g0 < w:
                        gw = min(512, w - g0)
                        pg = pg_pool.tile([D, 512], f32, tag="pg")
                        nc.tensor.matmul(pg[:, :gw], lhsT=WaW[:],
                                         rhs=xc[:, g0:g0 + gw], start=True, stop=True)
                        sT = w1pool.tile([D, 512], bf16, tag="sT")
                        nc.scalar.activation(sT[:, :gw], pg[:, :gw], AF.Sigmoid,
                                             bias=WaB[:, 0:1])
                        nc.vector.tensor_tensor(out=hTp[:, off + g0:off + g0 + gw],
                                                in0=xc[:, g0:g0 + gw],
                                                in1=sT[:, :gw], op=mybir.AluOpType.mult)
                        g0 += gw
                    off += w

            def ph1_flush(hTp, h_t, t_base, cn, use_xbar):
                """Transpose hTp[:, :cn] into node-major tiles and DMA to
                h_t[:, t_base:t_base+cn/128, :] (partition-major layout)."""
                nfull = cn // 128
                rem = cn - nfull * 128
                if nfull and use_xbar:
                    hst = hspool.tile([128, PIECE // 128, 128], bf16, tag="hst")
                    nc.sync.dma_start_transpose(hst[:, :nfull, :], hTp[:, :nfull * 128])
                    nc.sync.dma_start(out=h_t[:, t_base:t_base + nfull, :],
                                      in_=hst[:, :nfull, :])
                elif nfull:
                    # PE-transpose path (avoids xbar mode-serialization of the
                    # DMA stream while gathers are in flight)
                    hst = hspool.tile([128, PIECE // 128, 128], bf16, tag="hst")
                    t0 = 0
                    while t0 < nfull:
                        tn = min(4, nfull - t0)
                        pt = pt_pool.tile([128, 512], bf16, tag="pt")
                        for q in range(tn):
                            nc.tensor.transpose(
                                pt[:, q * 128:(q + 1) * 128],
                                hTp[:, (t0 + q) * 128:(t0 + q + 1) * 128], ident[:])
                        nc.vector.tensor_copy(
                            out=hst[:, t0:t0 + tn, :].rearrange("p t d -> p (t d)"),
                            in_=pt[:, :tn * 128])
                        t0 += tn
                    nc.sync.dma_start(out=h_t[:, t_base:t_base + nfull, :],
                                      in_=hst[:, :nfull, :])
                if rem:
                    pt = pt_pool.tile([128, 512], bf16, tag="pt")
                    nc.tensor.transpose(pt[:rem, :128], hTp[:, nfull * 128:nfull * 128 + rem],
                                        ident[:])
                    tl = w1pool.tile([128, 128], bf16, tag="tail")
                    nc.vector.tensor_copy(out=tl[:rem, :], in_=pt[:rem, :128])
                    nc.sync.dma_start(
                        out=h_t[:rem, t_base + nfull:t_base + nfull + 1, :],
                        in_=tl[:rem, :].rearrange("p (t d) -> p t d", t=1))

            # h1: nodes [0, H)
            base = 0
            while base < H:
                cn = min(PIECE, H - base)
                hTp = htpool.tile([D, PIECE], bf16, tag="hT")
                ph1_compute(hTp, base, cn)
                ph1_flush(hTp, h1_d, base // 128, cn, use_xbar=True)
                base += cn
            # h2: nodes [H, N)  (PE transposes: keep the DMA stream xbar-free
            # while pass-A gathers are running)
            while base < N:
                cn = min(PIECE, N - base)
                hTp = htpool.tile([D, PIECE], bf16, tag="hT")
                ph1_compute(hTp, base, cn)
                ph1_flush(hTp, h2_d, (base - H) // 128, cn, use_xbar=False)
                base += cn

            # ---- phase 2: two passes (A from h1, B from h2), one-hot scatter
            nq_counter = [0]
            gA_tiles = [None] * (LAg // p.GCHUNK)
            gB_tiles = [None] * (LBg // p.GCHUNK)
            MGT = 4
            mA_tiles = [None] * (-(-SA // MGT))
            mB_tiles = [None] * (-(-SB // MGT))

            def ensure_chunk(tiles, which, ci):
                if tiles[ci] is not None:
                    return
                g = (gApool if which == "A" else gBpool).tile(
                    [128, p.GT, D], bf16, tag="g" + which)
                idx_sb = idxA_sb if which == "A" else idxB_sb
                src = h1_rows if which == "A" else h2_rows
                c0 = ci * (p.GCHUNK // 16)
                nc.gpsimd.dma_gather(
                    out_ap=g[:], in_ap=src, idxs_ap=idx_sb[:, c0:c0 + p.GCHUNK // 16],
                    num_idxs=p.GCHUNK, num_idxs_reg=p.GCHUNK, elem_size=D,
                    queue_num=nq_counter[0] % p.NQ)
                nq_counter[0] += 1
                tiles[ci] = g

            def ensure_mchunk(tiles, which, ci):
                """Build 4 one-hot tiles on-chip: M[e,d] = (lr[e] == d)."""
                if tiles[ci] is not None:
                    return
                lr_sb = lrA_sb if which == "A" else lrB_sb
                ntile = SA if which == "A" else SB
                t0 = ci * MGT
                tn = min(MGT, ntile - t0)
                mt = mpool.tile([128, MGT, D], bf16, tag="m" + which)
                nc.vector.tensor_tensor(
                    out=mt[:, :tn, :],
                    in0=piota[:, :tn * 128].rearrange("p (t d) -> p t d", d=128),
                    in1=lr_sb[:, t0:t0 + tn].broadcast_to([128, tn, 128]),
                    op=mybir.AluOpType.is_equal)
                tiles[ci] = mt

            posA = np.concatenate([[0], np.cumsum(tA)]).astype(int)
            posB = np.concatenate([[0], np.cumsum(tB)]).astype(int)

            def scatter_pass(tcnts, pos, gtiles, mtiles, which, aggpool):
                """One-hot matmul scatter for one stream; returns agg tiles."""
                aggs = []
                b0 = 0
                while b0 < NBLK:
                    gn = min(4, NBLK - b0)
                    pa = pa_pool.tile([D, 512], f32, tag="pa")
                    for q in range(gn):
                        b = b0 + q
                        tcnt = int(tcnts[b])
                        for j in range(tcnt):
                            g = pos[b] + j
                            ensure_chunk(gtiles, which, g // p.GT)
                            ensure_mchunk(mtiles, which, g // MGT)
                            neigh = gtiles[g // p.GT][:, g % p.GT, :]
                            M = mtiles[g // MGT][:, g % MGT, :]
                            nc.tensor.matmul(pa[:, q * 128:(q + 1) * 128],
                                             lhsT=neigh, rhs=M,
                                             start=(j == 0), stop=(j == tcnt - 1))
                    agg = aggpool.tile([D, 512], bf16, tag="agg" + which)
                    nc.vector.tensor_copy(out=agg[:], in_=pa[:])
                    aggs.append(agg)
                    b0 += gn
                return aggs

            aggsA = scatter_pass(tA, posA, gA_tiles, mA_tiles, "A", aggApool)

            # ---- pass B + phase 3 fused per 4-block group
            OCH = 8
            ost = None
            ost_base = 0
            ost_n = 0
            b0 = 0
            while b0 < NBLK:
                gn = min(4, NBLK - b0)
                gi = b0 // 4
                pa = pa_pool.tile([D, 512], f32, tag="pa")
                for q in range(gn):
                    b = b0 + q
                    tcnt = int(tB[b])
                    for j in range(tcnt):
                        g = posB[b] + j
                        ensure_chunk(gB_tiles, "B", g // p.GT)
                        ensure_mchunk(mB_tiles, "B", g // MGT)
                        neigh = gB_tiles[g // p.GT][:, g % p.GT, :]
                        M = mB_tiles[g // MGT][:, g % MGT, :]
                        nc.tensor.matmul(pa[:, q * 128:(q + 1) * 128],
                                         lhsT=neigh, rhs=M,
                                         start=(j == 0), stop=(j == tcnt - 1))
                aggB = aggBpool.tile([D, 512], bf16, tag="aggB")
                nc.vector.tensor_copy(out=aggB[:], in_=pa[:])

                po = po_pool.tile([128, 512], f32, tag="po")
                for q in range(gn):
                    b = b0 + q
                    nb = min(128, NB - b * 128)
                    sl = slice(q * 128, q * 128 + D)
                    nc.tensor.matmul(po[:nb, sl],
                                     lhsT=xT_own[:, b * 128:b * 128 + nb],
                                     rhs=WnW[:], start=True, stop=False)
                    nc.tensor.matmul(po[:nb, sl],
                                     lhsT=aggsA[gi][:, q * 128:q * 128 + nb],
                                     rhs=WgW[:], start=False, stop=False)
                    nc.tensor.matmul(po[:nb, sl],
                                     lhsT=aggB[:, q * 128:q * 128 + nb],
                                     rhs=WgW[:], start=False, stop=False)
                    nc.tensor.matmul(po[:nb, sl], lhsT=onesR[:1, :nb],
                                     rhs=biasR[:1, :], start=False, stop=True)
                for q in range(gn):
                    b = b0 + q
                    nb = min(128, NB - b * 128)
                    if ost is None:
                        ost = ospool.tile([128, OCH * D], f32, tag="ost")
                        ost_base = b
                        ost_n = 0
                    nc.scalar.activation(ost[:nb, ost_n * D:(ost_n + 1) * D],
                                         po[:nb, q * 128:q * 128 + D], AF.Tanh)
                    ost_n += 1
                    if (ost_n == OCH) or (b == NBLK - 1):
                        rows0 = ost_base * 128
                        nfull_o = ost_n if nb == 128 else ost_n - 1
                        if nfull_o:
                            nc.sync.dma_start(
                                out=out_d[rows0:rows0 + nfull_o * 128, :].rearrange(
                                    "(t p) d -> p t d", p=128),
                                in_=ost[:, :nfull_o * D].rearrange(
                                    "p (t d) -> p t d", d=D))
                        if nb != 128:
                            nc.sync.dma_start(
                                out=out_d[rows0 + nfull_o * 128:
                                          rows0 + nfull_o * 128 + nb, :],
                                in_=ost[:nb, nfull_o * D:nfull_o * D + D])
                        ost = None
                b0 += gn

    nc.compile()
    return nc


# ---------------------------------------------------------------- host entry

def _host_prep(p: P, x, edge_index, Wn_w, Wn_b, Wg_w, Wg_b, Wa_w, Wa_b):
    x = np.asarray(x, np.float32)
    xT = np.ascontiguousarray(x.T).astype(BF16)
    tA, tB, LA, LB, LAg, LBg, per_core = prep_edges(
        p, np.asarray(edge_index)[0], np.asarray(edge_index)[1])

    shared = {
        "xT": xT,
        "WaW": np.asarray(Wa_w, np.float32).astype(BF16),
        "WaB": np.asarray(Wa_b, np.float32).reshape(p.D, 1),
        "WnW": np.asarray(Wn_w, np.float32).astype(BF16),
        "WgW": np.asarray(Wg_w, np.float32).astype(BF16),
        "biasR": (np.asarray(Wn_b, np.float32)
                  + np.asarray(Wg_b, np.float32)).reshape(1, p.D).astype(BF16),
        "onesR": np.ones((1, p.D), BF16),
        "ident": np.eye(p.D, dtype=np.float32).astype(BF16),
        "iota4": np.tile(np.arange(128, dtype=np.float32), 4)[None, :]
                 .repeat(128, 0).astype(BF16),
    }
    in_maps = []
    for k in range(p.NCORES):
        m = dict(shared)
        m["xT_own"] = np.ascontiguousarray(xT[:, k * p.NB:(k + 1) * p.NB])
        pc = per_core[k]
        m["idxA"], m["idxB"] = pc["idxA"], pc["idxB"]
        m["lrA"], m["lrB"] = pc["lrA"], pc["lrB"]
        in_maps.append(m)
    return tA, tB, LA, LB, LAg, LBg, in_maps


TRACE = False      # set True (e.g. from test.py) to capture an NTFF profile
LAST = None        # last BassKernelResults, for profiling/inspection


def kernel(**inputs) -> np.ndarray:
    global LAST
    from concourse import bass_utils
    bass_utils.upload_artifacts = lambda tmpdir: "local://" + tmpdir

    p = P()
    tA, tB, LA, LB, LAg, LBg, in_maps = _host_prep(p, **inputs)
    nc = build(p, tA, tB, LA, LB, LAg, LBg)
    kw = dict(trace=True, trace_cores=list(range(p.NCORES))) if TRACE else {}
    res = bass_utils.run_bass_kernel_spmd(
        nc, in_maps, core_ids=list(range(p.NCORES)), **kw)
    LAST = res
    out = np.concatenate([res.results[k]["out"] for k in range(p.NCORES)], axis=0)
    return out.astype(np.float32)


# revision 10
# speedup vs baseline: 1.3268x; 1.0852x over previous
"""AttentiveFPConv GNN message-passing kernel for 8 Trainium2 NeuronCores.

Reference computation (all fp32):
    alpha = sigmoid(x[col] @ Wa_w + Wa_b)          # per-edge attention
    neigh = x[col] * alpha                          # per-edge message
    aggr  = segment_sum(neigh, row, N)              # per-node aggregation
    out   = tanh(x @ Wn_w + Wn_b + aggr @ Wg_w + Wg_b)

Key algebraic identity: alpha depends only on the source node, so
    h = x * sigmoid(x @ Wa_w + Wa_b)                # per-NODE tensor
    aggr[n] = sum_{e: row[e]=n} h[col[e]]           # gather + segment-sum

Sharding: destination-node sharding. Core k owns nodes [5000k, 5000(k+1))
and ALL edges targeting them (balanced: rows are uniform). No collective
needed: each core computes its own aggr and output slice.

Per-core pipeline:
  Phase 1: h = x*sigmoid(x@Wa+b) for ALL nodes (replicated). h stored in
           HBM partition-major as [128, T, D] so the store DMA has large
           contiguous per-partition runs (full DMA rate); the gather
           addresses node c at row (c%128)*T + c//128.
  Phase 2: dma_gather h[col] in destination-sorted edge order (4 SWDGE
           queues, 2048-idx chunks); segment-sum via one-hot matmuls
           accumulating aggr^T in PSUM per 128-node block. One-hot M is
           built ON-CHIP: DVE tensor_tensor is_equal of a PSUM-resident
           iota against the per-edge local-row stream (bcast AP), 4 tiles
           per op. (dma_gather indices are int16, so edges are split into
           two streams by col < HSPLIT, each gathered against its own
           rebased h tensor.)
  Phase 3: out = tanh(x@Wn + aggr@Wg + ones x bias) -- bias added by a
           rank-1 matmul into the same PSUM accumulation group.
"""

import numpy as np
import ml_dtypes

BF16 = ml_dtypes.bfloat16

# ---------------------------------------------------------------- parameters

class P:
    """Problem/kernel parameters (full-size defaults; shrinkable for tests)."""
    def __init__(self, N=40000, D=128, NCORES=8, HSPLIT=9984,
                 GCHUNK=512, PH1_CHUNK=2048, NQ=4, SCRATCH=16384,
                 PIECE=4992, BATCH_M=True):
        assert D == 128
        self.N, self.D, self.NCORES = N, D, NCORES
        self.NB = N // NCORES                 # nodes per core
        self.HSPLIT = HSPLIT                  # col split for int16 gather idx
        self.GCHUNK = GCHUNK                  # idxs per dma_gather call
        self.GT = GCHUNK // 128               # gather tiles per chunk
        self.PH1_CHUNK = PH1_CHUNK            # nodes per phase-1 xT chunk
        self.NBLK = (self.NB + 127) // 128    # 128-node blocks per core
        self.NQ = NQ                          # SWDGE queues for dma_gather
        self.SCRATCH = SCRATCH                # SWDGE desc ring bytes/partition
        self.PIECE = PIECE                    # nodes per phase-1 staging piece
        self.BATCH_M = BATCH_M                # build one-hot 4 tiles per DVE op
        self.T1 = HSPLIT // 128               # h1 tiles (partition-major)
        self.T2 = (N - HSPLIT + 127) // 128   # h2 tiles


# ------------------------------------------------------------ host edge prep

def prep_edges(p: P, row: np.ndarray, col: np.ndarray):
    """Per-core destination-sorted, block-padded edge streams."""
    row = np.asarray(row).astype(np.int64)
    col = np.asarray(col).astype(np.int64)
    cores = []
    for k in range(p.NCORES):
        sel = (row // p.NB) == k
        r = (row[sel] - k * p.NB).astype(np.int32)
        c = col[sel].astype(np.int32)
        order = np.argsort(r, kind="stable")
        r, c = r[order], c[order]
        lo = np.searchsorted(r, np.arange(p.NBLK) * 128)
        hi = np.searchsorted(r, np.minimum(np.arange(1, p.NBLK + 1) * 128, p.NB))
        blocks = []
        for b in range(p.NBLK):
            rb = r[lo[b]:hi[b]] - b * 128
            cb = c[lo[b]:hi[b]]
            mA = cb < p.HSPLIT
            cA = cb[mA]
            c2 = cb[~mA] - p.HSPLIT
            # partition-major h layout: node c -> row (c%128)*T + c//128
            gA = (cA % 128) * p.T1 + cA // 128
            gB = (c2 % 128) * p.T2 + c2 // 128
            blocks.append(((gA, rb[mA]), (gB, rb[~mA])))
        cores.append(blocks)

    nA = np.array([[len(cores[k][b][0][0]) for b in range(p.NBLK)]
                   for k in range(p.NCORES)])
    nB = np.array([[len(cores[k][b][1][0]) for b in range(p.NBLK)]
                   for k in range(p.NCORES)])
    tA = np.maximum(1, -(-nA.max(axis=0) // 128))          # [NBLK]
    tB = np.maximum(1, -(-nB.max(axis=0) // 128))

    LA, LB = int(tA.sum()) * 128, int(tB.sum()) * 128
    LAg = -(-LA // p.GCHUNK) * p.GCHUNK
    LBg = -(-LB // p.GCHUNK) * p.GCHUNK

    per_core = []
    for k in range(p.NCORES):
        idxA = np.zeros(LAg, np.int16); lrA = np.full(LA, -1.0, np.float32)
        idxB = np.zeros(LBg, np.int16); lrB = np.full(LB, -1.0, np.float32)
        oA = oB = 0
        for b in range(p.NBLK):
            (cA, rA), (cB, rB) = cores[k][b]
            idxA[oA:oA + len(cA)] = cA; lrA[oA:oA + len(rA)] = rA
            oA += int(tA[b]) * 128
            idxB[oB:oB + len(cB)] = cB; lrB[oB:oB + len(rB)] = rB
            oB += int(tB[b]) * 128
        per_core.append({
            "idxA": np.tile(idxA.reshape(-1, 16).T, (8, 1)),   # [128, LAg/16]
            "idxB": np.tile(idxB.reshape(-1, 16).T, (8, 1)),
            "lrA": lrA.reshape(-1, 128).T.copy(),              # [128, LA/128]
            "lrB": lrB.reshape(-1, 128).T.copy(),
        })
    return tA, tB, LA, LB, LAg, LBg, per_core


# ------------------------------------------------------------- device kernel

def build(p: P, tA, tB, LA, LB, LAg, LBg):
    from concourse import bacc, mybir, tile

    f32, bf16, i16 = mybir.dt.float32, mybir.dt.bfloat16, mybir.dt.int16
    AF = mybir.ActivationFunctionType
    nc = bacc.Bacc("TRN2", target_bir_lowering=False, debug=False,
                   num_devices=p.NCORES, num_swdge_queues=p.NQ,
                   dynamic_dma_scratch_size=p.SCRATCH)

    N, D, NB, NBLK = p.N, p.D, p.NB, p.NBLK
    H = p.HSPLIT                    # h1 nodes; h2 nodes = N - H
    T1, T2 = p.T1, p.T2
    PIECE = p.PIECE
    assert H % 128 == 0 and H % PIECE == 0
    SA, SB = LA // 128, LB // 128   # one-hot tiles per stream

    xT_d   = nc.dram_tensor("xT", [D, N], bf16, kind="ExternalInput")
    xTo_d  = nc.dram_tensor("xT_own", [D, NB], bf16, kind="ExternalInput")
    WaW_d  = nc.dram_tensor("WaW", [D, D], bf16, kind="ExternalInput")
    WaB_d  = nc.dram_tensor("WaB", [D, 1], f32, kind="ExternalInput")
    WnW_d  = nc.dram_tensor("WnW", [D, D], bf16, kind="ExternalInput")
    WgW_d  = nc.dram_tensor("WgW", [D, D], bf16, kind="ExternalInput")
    bias_d = nc.dram_tensor("biasR", [1, D], bf16, kind="ExternalInput")
    ones_d = nc.dram_tensor("onesR", [1, D], bf16, kind="ExternalInput")
    ident_d= nc.dram_tensor("ident", [D, D], bf16, kind="ExternalInput")
    iota_d = nc.dram_tensor("iota4", [128, 512], bf16, kind="ExternalInput")
    idxA_d = nc.dram_tensor("idxA", [128, LAg // 16], i16, kind="ExternalInput")
    idxB_d = nc.dram_tensor("idxB", [128, LBg // 16], i16, kind="ExternalInput")
    lrA_d  = nc.dram_tensor("lrA", [128, SA], f32, kind="ExternalInput")
    lrB_d  = nc.dram_tensor("lrB", [128, SB], f32, kind="ExternalInput")
    out_d  = nc.dram_tensor("out", [NB, D], f32, kind="ExternalOutput")
    h1_d   = nc.dram_tensor("h1", [128, T1, D], bf16, kind="Internal")
    h2_d   = nc.dram_tensor("h2", [128, T2, D], bf16, kind="Internal")
    h1_rows = h1_d[:, :, :].rearrange("p t d -> (p t) d")
    h2_rows = h2_d[:, :, :].rearrange("p t d -> (p t) d")

    with tile.TileContext(nc) as tc:
        with (
            tc.tile_pool(name="const", bufs=1) as cpool,
            tc.tile_pool(name="xchunk", bufs=3) as xpool,
            tc.tile_pool(name="hT", bufs=2) as htpool,
            tc.tile_pool(name="hstage", bufs=1) as hspool,
            tc.tile_pool(name="pg", bufs=2, space="PSUM") as pg_pool,
            tc.tile_pool(name="pt", bufs=1, space="PSUM") as pt_pool,
            tc.tile_pool(name="pi", bufs=1, space="PSUM") as pi_pool,
            tc.tile_pool(name="pa", bufs=2, space="PSUM") as pa_pool,
            tc.tile_pool(name="po", bufs=2, space="PSUM") as po_pool,
            tc.tile_pool(name="sA", bufs=16) as gApool,
            tc.tile_pool(name="sB", bufs=16) as gBpool,
            tc.tile_pool(name="m", bufs=6) as mpool,
            tc.tile_pool(name="aggA", bufs=(NBLK + 3) // 4) as aggApool,
            tc.tile_pool(name="aggB", bufs=3) as aggBpool,
            tc.tile_pool(name="ph1w", bufs=4) as w1pool,
            tc.tile_pool(name="ostage", bufs=2) as ospool,
        ):
            # ---- constants into SBUF
            WaW = cpool.tile([D, D], bf16); nc.sync.dma_start(out=WaW[:], in_=WaW_d[:])
            WaB = cpool.tile([D, 1], f32); nc.sync.dma_start(out=WaB[:], in_=WaB_d[:])
            WnW = cpool.tile([D, D], bf16); nc.sync.dma_start(out=WnW[:], in_=WnW_d[:])
            WgW = cpool.tile([D, D], bf16); nc.sync.dma_start(out=WgW[:], in_=WgW_d[:])
            biasR = cpool.tile([1, D], bf16); nc.sync.dma_start(out=biasR[:], in_=bias_d[:])
            onesR = cpool.tile([1, D], bf16); nc.sync.dma_start(out=onesR[:], in_=ones_d[:])
            ident = cpool.tile([D, D], bf16); nc.sync.dma_start(out=ident[:], in_=ident_d[:])
            iota4 = cpool.tile([128, 512], bf16); nc.sync.dma_start(out=iota4[:], in_=iota_d[:])
            xT_own = cpool.tile([D, NB], bf16); nc.sync.dma_start(out=xT_own[:], in_=xTo_d[:])
            lrA_sb = cpool.tile([128, SA], f32); nc.sync.dma_start(out=lrA_sb[:], in_=lrA_d[:])
            lrB_sb = cpool.tile([128, SB], f32); nc.sync.dma_start(out=lrB_sb[:], in_=lrB_d[:])
            idxA_sb = cpool.tile([128, LAg // 16], i16)
            nc.sync.dma_start(out=idxA_sb[:], in_=idxA_d[:])
            idxB_sb = cpool.tile([128, LBg // 16], i16)
            nc.sync.dma_start(out=idxB_sb[:], in_=idxB_d[:])

            # PSUM-resident iota (values 0..127 x4), built once via matmul
            piota = pi_pool.tile([128, 512], f32)
            nc.tensor.matmul(piota[:], lhsT=ident[:], rhs=iota4[:],
                             start=True, stop=True)

            # ---- phase 1: h = x * sigmoid(x@Wa + b); hT pieces -> h[128,T,D]
            def ph1_compute(hTp, base, cn):
                """Compute hT for nodes [base, base+cn) into hTp[:, :cn]."""
                off = 0
                while off < cn:
                    w = min(p.PH1_CHUNK, cn - off)
                    xc = xpool.tile([D, p.PH1_CHUNK], bf16, tag="xc")
                    nc.sync.dma_start(out=xc[:, :w], in_=xT_d[:, base + off:base + off + w])
                    g0 = 0
                    while g0 < w:
                        gw = min(512, w - g0)
                        pg = pg_pool.tile([D, 512], f32, tag="pg")
                        nc.tensor.matmul(pg[:, :gw], lhsT=WaW[:],
                                         rhs=xc[:, g0:g0 + gw], start=True, stop=True)
                        sT = w1pool.tile([D, 512], bf16, tag="sT")
                        nc.scalar.activation(sT[:, :gw], pg[:, :gw], AF.Sigmoid,
                                             bias=WaB[:, 0:1])
                        nc.vector.tensor_tensor(out=hTp[:, off + g0:off + g0 + gw],
                                                in0=xc[:, g0:g0 + gw],
                                                in1=sT[:, :gw], op=mybir.AluOpType.mult)
                        g0 += gw
                    off += w

            def ph1_flush(hTp, h_t, t_base, cn, use_xbar):
                """Transpose hTp[:, :cn] into node-major tiles and DMA to
                h_t[:, t_base:t_base+cn/128, :] (partition-major layout)."""
                nfull = cn // 128
                rem = cn - nfull * 128
                if nfull and use_xbar:
                    hst = hspool.tile([128, PIECE // 128, 128], bf16, tag="hst")
                    nc.sync.dma_start_transpose(hst[:, :nfull, :], hTp[:, :nfull * 128])
                    nc.sync.dma_start(out=h_t[:, t_base:t_base + nfull, :],
                                      in_=hst[:, :nfull, :])
                elif nfull:
                    # PE-transpose path (avoids xbar mode-serialization of the
                    # DMA stream while gathers are in flight)
                    hst = hspool.tile([128, PIECE // 128, 128], bf16, tag="hst")
                    t0 = 0
                    while t0 < nfull:
                        tn = min(4, nfull - t0)
                        pt = pt_pool.tile([128, 512], bf16, tag="pt")
                        for q in range(tn):
                            nc.tensor.transpose(
                                pt[:, q * 128:(q + 1) * 128],
                                hTp[:, (t0 + q) * 128:(t0 + q + 1) * 128], ident[:])
                        nc.vector.tensor_copy(
                            out=hst[:, t0:t0 + tn, :].rearrange("p t d -> p (t d)"),
                            in_=pt[:, :tn * 128])
                        t0 += tn
                    nc.sync.dma_start(out=h_t[:, t_base:t_base + nfull, :],
                                      in_=hst[:, :nfull, :])
                if rem:
                    pt = pt_pool.tile([128, 512], bf16, tag="pt")
                    nc.tensor.transpose(pt[:rem, :128], hTp[:, nfull * 128:nfull * 128 + rem],
                                        ident[:])
                    tl = w1pool.tile([128, 128], bf16, tag="tail")
                    nc.vector.tensor_copy(out=tl[:rem, :], in_=pt[:rem, :128])
                    nc.sync.dma_start(
                        out=h_t[:rem, t_base + nfull:t_base + nfull + 1, :],
                        in_=tl[:rem, :].rearrange("p (t d) -> p t d", t=1))

            # h1: nodes [0, H)
            base = 0
            while base < H:
                cn = min(PIECE, H - base)
                hTp = htpool.tile([D, PIECE], bf16, tag="hT")
                ph1_compute(hTp, base, cn)
                ph1_flush(hTp, h1_d, base // 128, cn, use_xbar=True)
                base += cn
            # h2: nodes [H, N)  (PE transposes: keep the DMA stream xbar-free
            # while pass-A gathers are running)
            while base < N:
                cn = min(PIECE, N - base)
                hTp = htpool.tile([D, PIECE], bf16, tag="hT")
                ph1_compute(hTp, base, cn)
                ph1_flush(hTp, h2_d, (base - H) // 128, cn, use_xbar=False)
                base += cn

            # ---- phase 2: two passes (A from h1, B from h2), one-hot scatter
            nq_counter = [0]
            gA_tiles = [None] * (LAg // p.GCHUNK)
            gB_tiles = [None] * (LBg // p.GCHUNK)
            MGT = 4
            mA_tiles = [None] * (-(-SA // MGT))
            mB_tiles = [None] * (-(-SB // MGT))

            def ensure_chunk(tiles, which, ci):
                if tiles[ci] is not None:
                    return
                g = (gApool if which == "A" else gBpool).tile(
                    [128, p.GT, D], bf16, tag="g" + which)
                idx_sb = idxA_sb if which == "A" else idxB_sb
                src = h1_rows if which == "A" else h2_rows
                c0 = ci * (p.GCHUNK // 16)
                nc.gpsimd.dma_gather(
                    out_ap=g[:], in_ap=src, idxs_ap=idx_sb[:, c0:c0 + p.GCHUNK // 16],
                    num_idxs=p.GCHUNK, num_idxs_reg=p.GCHUNK, elem_size=D,
                    queue_num=nq_counter[0] % p.NQ)
                nq_counter[0] += 1
                tiles[ci] = g

            def ensure_mchunk(tiles, which, ci):
                """Build 4 one-hot tiles on-chip: M[e,d] = (lr[e] == d)."""
                if tiles[ci] is not None:
                    return
                lr_sb = lrA_sb if which == "A" else lrB_sb
                ntile = SA if which == "A" else SB
                t0 = ci * MGT
                tn = min(MGT, ntile - t0)
                mt = mpool.tile([128, MGT, D], bf16, tag="m" + which)
                nc.vector.tensor_tensor(
                    out=mt[:, :tn, :],
                    in0=piota[:, :tn * 128].rearrange("p (t d) -> p t d", d=128),
                    in1=lr_sb[:, t0:t0 + tn].broadcast_to([128, tn, 128]),
                    op=mybir.AluOpType.is_equal)
                tiles[ci] = mt

            posA = np.concatenate([[0], np.cumsum(tA)]).astype(int)
            posB = np.concatenate([[0], np.cumsum(tB)]).astype(int)

            def scatter_pass(tcnts, pos, gtiles, mtiles, which, aggpool):
                """One-hot matmul scatter for one stream; returns agg tiles."""
                aggs = []
                b0 = 0
                while b0 < NBLK:
                    gn = min(4, NBLK - b0)
                    pa = pa_pool.tile([D, 512], f32, tag="pa")
                    for q in range(gn):
                        b = b0 + q
                        tcnt = int(tcnts[b])
                        for j in range(tcnt):
                            g = pos[b] + j
                            ensure_chunk(gtiles, which, g // p.GT)
                            ensure_mchunk(mtiles, which, g // MGT)
                            neigh = gtiles[g // p.GT][:, g % p.GT, :]
                            M = mtiles[g // MGT][:, g % MGT, :]
                            nc.tensor.matmul(pa[:, q * 128:(q + 1) * 128],
                                             lhsT=neigh, rhs=M,
                                             start=(j == 0), stop=(j == tcnt - 1))
                    agg = aggpool.tile([D, 512], bf16, tag="agg" + which)
                    nc.vector.tensor_copy(out=agg[:], in_=pa[:])
                    aggs.append(agg)
                    b0 += gn
                return aggs

            aggsA = scatter_pass(tA, posA, gA_tiles, mA_tiles, "A", aggApool)

            # ---- pass B + phase 3 fused per 4-block group
            OCH = 8
            ost = None
            ost_base = 0
            ost_n = 0
            b0 = 0
            while b0 < NBLK:
                gn = min(4, NBLK - b0)
                gi = b0 // 4
                pa = pa_pool.tile([D, 512], f32, tag="pa")
                for q in range(gn):
                    b = b0 + q
                    tcnt = int(tB[b])
                    for j in range(tcnt):
                        g = posB[b] + j
                        ensure_chunk(gB_tiles, "B", g // p.GT)
                        ensure_mchunk(mB_tiles, "B", g // MGT)
                        neigh = gB_tiles[g // p.GT][:, g % p.GT, :]
                        M = mB_tiles[g // MGT][:, g % MGT, :]
                        nc.tensor.matmul(pa[:, q * 128:(q + 1) * 128],
                                         lhsT=neigh, rhs=M,
                                         start=(j == 0), stop=(j == tcnt - 1))
                aggB = aggBpool.tile([D, 512], bf16, tag="aggB")
                nc.vector.tensor_copy(out=aggB[:], in_=pa[:])

                po = po_pool.tile([128, 512], f32, tag="po")
                for q in range(gn):
                    b = b0 + q
                    nb = min(128, NB - b * 128)
                    sl = slice(q * 128, q * 128 + D)
                    nc.tensor.matmul(po[:nb, sl],
                                     lhsT=xT_own[:, b * 128:b * 128 + nb],
                                     rhs=WnW[:], start=True, stop=False)
                    nc.tensor.matmul(po[:nb, sl],
                                     lhsT=aggsA[gi][:, q * 128:q * 128 + nb],
                                     rhs=WgW[:], start=False, stop=False)
                    nc.tensor.matmul(po[:nb, sl],
                                     lhsT=aggB[:, q * 128:q * 128 + nb],
                                     rhs=WgW[:], start=False, stop=False)
                    nc.tensor.matmul(po[:nb, sl], lhsT=onesR[:1, :nb],
                                     rhs=biasR[:1, :], start=False, stop=True)
                for q in range(gn):
                    b = b0 + q
                    nb = min(128, NB - b * 128)
                    if ost is None:
                        ost = ospool.tile([128, OCH * D], f32, tag="ost")
                        ost_base = b
                        ost_n = 0
                    nc.scalar.activation(ost[:nb, ost_n * D:(ost_n + 1) * D],
                                         po[:nb, q * 128:q * 128 + D], AF.Tanh)
                    ost_n += 1
                    if (ost_n == OCH) or (b == NBLK - 1):
                        rows0 = ost_base * 128
                        nfull_o = ost_n if nb == 128 else ost_n - 1
                        if nfull_o:
                            nc.sync.dma_start(
                                out=out_d[rows0:rows0 + nfull_o * 128, :].rearrange(
                                    "(t p) d -> p t d", p=128),
                                in_=ost[:, :nfull_o * D].rearrange(
                                    "p (t d) -> p t d", d=D))
                        if nb != 128:
                            nc.sync.dma_start(
                                out=out_d[rows0 + nfull_o * 128:
                                          rows0 + nfull_o * 128 + nb, :],
                                in_=ost[:nb, nfull_o * D:nfull_o * D + D])
                        ost = None
                b0 += gn

    nc.compile()
    return nc


# ---------------------------------------------------------------- host entry

def _host_prep(p: P, x, edge_index, Wn_w, Wn_b, Wg_w, Wg_b, Wa_w, Wa_b):
    x = np.asarray(x, np.float32)
    xT = np.ascontiguousarray(x.T).astype(BF16)
    tA, tB, LA, LB, LAg, LBg, per_core = prep_edges(
        p, np.asarray(edge_index)[0], np.asarray(edge_index)[1])

    shared = {
        "xT": xT,
        "WaW": np.asarray(Wa_w, np.float32).astype(BF16),
        "WaB": np.asarray(Wa_b, np.float32).reshape(p.D, 1),
        "WnW": np.asarray(Wn_w, np.float32).astype(BF16),
        "WgW": np.asarray(Wg_w, np.float32).astype(BF16),
        "biasR": (np.asarray(Wn_b, np.float32)
                  + np.asarray(Wg_b, np.float32)).reshape(1, p.D).astype(BF16),
        "onesR": np.ones((1, p.D), BF16),
        "ident": np.eye(p.D, dtype=np.float32).astype(BF16),
        "iota4": np.tile(np.arange(128, dtype=np.float32), 4)[None, :]
                 .repeat(128, 0).astype(BF16),
    }
    in_maps = []
    for k in range(p.NCORES):
        m = dict(shared)
        m["xT_own"] = np.ascontiguousarray(xT[:, k * p.NB:(k + 1) * p.NB])
        pc = per_core[k]
        m["idxA"], m["idxB"] = pc["idxA"], pc["idxB"]
        m["lrA"], m["lrB"] = pc["lrA"], pc["lrB"]
        in_maps.append(m)
    return tA, tB, LA, LB, LAg, LBg, in_maps


TRACE = False      # set True (e.g. from test.py) to capture an NTFF profile
LAST = None        # last BassKernelResults, for profiling/inspection


def kernel(**inputs) -> np.ndarray:
    global LAST
    from concourse import bass_utils
    bass_utils.upload_artifacts = lambda tmpdir: "local://" + tmpdir

    p = P()
    tA, tB, LA, LB, LAg, LBg, in_maps = _host_prep(p, **inputs)
    nc = build(p, tA, tB, LA, LB, LAg, LBg)
    kw = dict(trace=True, trace_cores=list(range(p.NCORES))) if TRACE else {}
    res = bass_utils.run_bass_kernel_spmd(
        nc, in_maps, core_ids=list(range(p.NCORES)), **kw)
    LAST = res
    out = np.concatenate([res.results[k]["out"] for k in range(p.NCORES)], axis=0)
    return out.astype(np.float32)


# revision 12
# speedup vs baseline: 1.5745x; 1.1868x over previous
"""AttentiveFPConv GNN message-passing kernel for 8 Trainium2 NeuronCores.

Reference computation (all fp32):
    alpha = sigmoid(x[col] @ Wa_w + Wa_b)          # per-edge attention
    neigh = x[col] * alpha                          # per-edge message
    aggr  = segment_sum(neigh, row, N)              # per-node aggregation
    out   = tanh(x @ Wn_w + Wn_b + aggr @ Wg_w + Wg_b)

Key algebraic identity: alpha depends only on the source node, so
    h = x * sigmoid(x @ Wa_w + Wa_b)                # per-NODE tensor
    aggr[n] = sum_{e: row[e]=n} h[col[e]]           # gather + segment-sum

Sharding: destination-node sharding. Core k owns nodes [5000k, 5000(k+1))
and ALL edges targeting them (balanced: rows are uniform). No collective
needed: each core computes its own aggr and output slice.

Per-core pipeline:
  Phase 1: h = x*sigmoid(x@Wa+b) for ALL nodes (replicated). h stored in
           HBM partition-major as [128, T, D] so the store DMA has large
           contiguous per-partition runs (full DMA rate); the gather
           addresses node c at row (c%128)*T + c//128.
  Phase 2: dma_gather h[col] in destination-sorted edge order (4 SWDGE
           queues, 512-idx chunks: the 1024-desc ring then pipelines two
           chunks per queue); segment-sum via one-hot matmuls accumulating
           aggr^T in PSUM per 128-node block. One-hot M is built ON-CHIP:
           DVE tensor_tensor is_equal of a bf16 iota against the per-edge
           local-row stream (bcast AP), 4 tiles per op.
           Edges are split into THREE streams by col range (h segments
           written in order), so gathers for stream s can start as soon
           as segment s is in HBM -- keeps the Pool engine (the SWDGE
           descriptor generator, the critical resource) continuously fed.
           Streams also keep every int16 gather index < 32768.
  Phase 3: out = tanh(x@Wn + aggr@Wg + ones x bias) -- bias added by a
           rank-1 matmul into the same PSUM accumulation group.
"""

import numpy as np
import ml_dtypes

BF16 = ml_dtypes.bfloat16

# ---------------------------------------------------------------- parameters

class P:
    """Problem/kernel parameters (full-size defaults; shrinkable for tests)."""
    def __init__(self, N=40000, D=128, NCORES=8, SPLITS=(9984, 24960),
                 GCHUNK=512, PH1_CHUNK=2048, NQ=4, SCRATCH=16384,
                 PIECE=4992):
        assert D == 128
        self.N, self.D, self.NCORES = N, D, NCORES
        self.NB = N // NCORES                 # nodes per core
        self.GCHUNK = GCHUNK                  # idxs per dma_gather call
        self.GT = GCHUNK // 128               # gather tiles per chunk
        self.PH1_CHUNK = PH1_CHUNK            # nodes per phase-1 xT chunk
        self.NBLK = (self.NB + 127) // 128    # 128-node blocks per core
        self.NQ = NQ                          # SWDGE queues for dma_gather
        self.SCRATCH = SCRATCH                # SWDGE desc ring bytes/partition
        self.PIECE = PIECE                    # nodes per phase-1 staging piece
        # h segments: [lo, hi) node ranges, each with partition-major tiles
        bounds = (0,) + tuple(SPLITS) + (N,)
        self.SEG = []
        for lo, hi in zip(bounds[:-1], bounds[1:]):
            T = (hi - lo + 127) // 128
            assert 127 * T + (T - 1) < 32768, "int16 gather index overflow"
            self.SEG.append((lo, hi, T))
        self.NSEG = len(self.SEG)


# ------------------------------------------------------------ host edge prep

def prep_edges(p: P, row: np.ndarray, col: np.ndarray):
    """Per-core destination-sorted, block-padded edge streams (one per h
    segment). Returns per-stream tile counts and per-core idx/lr arrays."""
    row = np.asarray(row).astype(np.int64)
    col = np.asarray(col).astype(np.int64)
    NS = p.NSEG
    cores = []
    for k in range(p.NCORES):
        sel = (row // p.NB) == k
        r = (row[sel] - k * p.NB).astype(np.int32)
        c = col[sel].astype(np.int32)
        order = np.argsort(r, kind="stable")
        r, c = r[order], c[order]
        lo = np.searchsorted(r, np.arange(p.NBLK) * 128)
        hi = np.searchsorted(r, np.minimum(np.arange(1, p.NBLK + 1) * 128, p.NB))
        blocks = []
        for b in range(p.NBLK):
            rb = r[lo[b]:hi[b]] - b * 128
            cb = c[lo[b]:hi[b]]
            per_stream = []
            for (slo, shi, T) in p.SEG:
                m = (cb >= slo) & (cb < shi)
                cs = cb[m] - slo
                # partition-major h layout: node c -> row (c%128)*T + c//128
                per_stream.append(((cs % 128) * T + cs // 128, rb[m]))
            blocks.append(per_stream)
        cores.append(blocks)

    cnt = np.array([[[len(cores[k][b][s][0]) for b in range(p.NBLK)]
                     for s in range(NS)] for k in range(p.NCORES)])
    tS = np.maximum(1, -(-cnt.max(axis=0) // 128))          # [NS, NBLK]
    L = [int(tS[s].sum()) * 128 for s in range(NS)]
    Lg = [-(-L[s] // p.GCHUNK) * p.GCHUNK for s in range(NS)]

    per_core = []
    for k in range(p.NCORES):
        m = {}
        for s in range(NS):
            idx = np.zeros(Lg[s], np.int16)
            lr = np.full(L[s], -1.0, np.float32)
            o = 0
            for b in range(p.NBLK):
                cs, rs = cores[k][b][s]
                idx[o:o + len(cs)] = cs
                lr[o:o + len(rs)] = rs
                o += int(tS[s][b]) * 128
            m[f"idx{s}"] = np.tile(idx.reshape(-1, 16).T, (8, 1))
            m[f"lr{s}"] = lr.reshape(-1, 128).T.astype(BF16)
        per_core.append(m)
    return tS, L, Lg, per_core


# ------------------------------------------------------------- device kernel

def build(p: P, tS, L, Lg):
    from concourse import bacc, mybir, tile

    f32, bf16, i16 = mybir.dt.float32, mybir.dt.bfloat16, mybir.dt.int16
    AF = mybir.ActivationFunctionType
    nc = bacc.Bacc("TRN2", target_bir_lowering=False, debug=False,
                   num_devices=p.NCORES, num_swdge_queues=p.NQ,
                   dynamic_dma_scratch_size=p.SCRATCH)

    N, D, NB, NBLK, NS = p.N, p.D, p.NB, p.NBLK, p.NSEG
    PIECE = p.PIECE
    S = [L[s] // 128 for s in range(NS)]      # one-hot tiles per stream

    xT_d   = nc.dram_tensor("xT", [D, N], bf16, kind="ExternalInput")
    xTo_d  = nc.dram_tensor("xT_own", [D, NB], bf16, kind="ExternalInput")
    WaW_d  = nc.dram_tensor("WaW", [D, D], bf16, kind="ExternalInput")
    WaB_d  = nc.dram_tensor("WaB", [D, 1], f32, kind="ExternalInput")
    WnW_d  = nc.dram_tensor("WnW", [D, D], bf16, kind="ExternalInput")
    WgW_d  = nc.dram_tensor("WgW", [D, D], bf16, kind="ExternalInput")
    bias_d = nc.dram_tensor("biasR", [1, D], bf16, kind="ExternalInput")
    ones_d = nc.dram_tensor("onesR", [1, D], bf16, kind="ExternalInput")
    ident_d= nc.dram_tensor("ident", [D, D], bf16, kind="ExternalInput")
    iota_d = nc.dram_tensor("iota4", [128, 512], bf16, kind="ExternalInput")
    idx_d  = [nc.dram_tensor(f"idx{s}", [128, Lg[s] // 16], i16,
                             kind="ExternalInput") for s in range(NS)]
    lr_d   = [nc.dram_tensor(f"lr{s}", [128, S[s]], bf16,
                             kind="ExternalInput") for s in range(NS)]
    out_d  = nc.dram_tensor("out", [NB, D], f32, kind="ExternalOutput")
    h_d    = [nc.dram_tensor(f"h{s}", [128, p.SEG[s][2], D], bf16,
                             kind="Internal") for s in range(NS)]
    h_rows = [h_d[s][:, :, :].rearrange("p t d -> (p t) d") for s in range(NS)]

    with tile.TileContext(nc) as tc:
        with (
            tc.tile_pool(name="const", bufs=1) as cpool,
            tc.tile_pool(name="xchunk", bufs=3) as xpool,
            tc.tile_pool(name="hT", bufs=2) as htpool,
            tc.tile_pool(name="hstage", bufs=2) as hspool,
            tc.tile_pool(name="pg", bufs=3, space="PSUM") as pg_pool,
            tc.tile_pool(name="pt", bufs=1, space="PSUM") as pt_pool,
            tc.tile_pool(name="pa", bufs=2, space="PSUM") as pa_pool,
            tc.tile_pool(name="po", bufs=2, space="PSUM") as po_pool,
            tc.tile_pool(name="s0", bufs=16) as g0pool,
            tc.tile_pool(name="s1", bufs=16) as g1pool,
            tc.tile_pool(name="s2", bufs=16) as g2pool,
            tc.tile_pool(name="m", bufs=8) as mpool,
            tc.tile_pool(name="agg0", bufs=(NBLK + 3) // 4) as agg0pool,
            tc.tile_pool(name="agg1", bufs=(NBLK + 3) // 4) as agg1pool,
            tc.tile_pool(name="agg2", bufs=3) as agg2pool,
            tc.tile_pool(name="ph1w", bufs=4) as w1pool,
            tc.tile_pool(name="ostage", bufs=2) as ospool,
        ):
            gpools = [g0pool, g1pool, g2pool]
            aggpools = [agg0pool, agg1pool, agg2pool]

            # ---- constants needed by phase 1 (issued first on the SP queue)
            WaW = cpool.tile([D, D], bf16); nc.sync.dma_start(out=WaW[:], in_=WaW_d[:])
            WaB = cpool.tile([D, 1], f32); nc.sync.dma_start(out=WaB[:], in_=WaB_d[:])
            ident = cpool.tile([D, D], bf16); nc.sync.dma_start(out=ident[:], in_=ident_d[:])

            # phase-2/3 constants: issued on the Act HWDGE queue so they do
            # not delay the phase-1 x-chunk stream on the SP queue.
            WnW = cpool.tile([D, D], bf16); nc.scalar.dma_start(out=WnW[:], in_=WnW_d[:])
            WgW = cpool.tile([D, D], bf16); nc.scalar.dma_start(out=WgW[:], in_=WgW_d[:])
            biasR = cpool.tile([1, D], bf16); nc.scalar.dma_start(out=biasR[:], in_=bias_d[:])
            onesR = cpool.tile([1, D], bf16); nc.scalar.dma_start(out=onesR[:], in_=ones_d[:])
            iota4 = cpool.tile([128, 512], bf16); nc.scalar.dma_start(out=iota4[:], in_=iota_d[:])
            xT_own = cpool.tile([D, NB], bf16); nc.scalar.dma_start(out=xT_own[:], in_=xTo_d[:])
            lr_sb = []
            idx_sb = []
            for s in range(NS):
                lt = cpool.tile([128, S[s]], bf16, tag=f"lr{s}_sb")
                nc.scalar.dma_start(out=lt[:], in_=lr_d[s][:])
                lr_sb.append(lt)
                it = cpool.tile([128, Lg[s] // 16], i16, tag=f"idx{s}_sb")
                nc.scalar.dma_start(out=it[:], in_=idx_d[s][:])
                idx_sb.append(it)

            # ---- phase 1: h = x * sigmoid(x@Wa + b); hT pieces -> h[128,T,D]
            def ph1_compute(hTp, base, cn):
                """Compute hT for nodes [base, base+cn) into hTp[:, :cn]."""
                off = 0
                while off < cn:
                    w = min(p.PH1_CHUNK, cn - off)
                    xc = xpool.tile([D, p.PH1_CHUNK], bf16, tag="xc")
                    nc.sync.dma_start(out=xc[:, :w], in_=xT_d[:, base + off:base + off + w])
                    g0 = 0
                    while g0 < w:
                        gw = min(512, w - g0)
                        pg = pg_pool.tile([D, 512], f32, tag="pg")
                        nc.tensor.matmul(pg[:, :gw], lhsT=WaW[:],
                                         rhs=xc[:, g0:g0 + gw], start=True, stop=True)
                        sT = w1pool.tile([D, 512], bf16, tag="sT")
                        nc.scalar.activation(sT[:, :gw], pg[:, :gw], AF.Sigmoid,
                                             bias=WaB[:, 0:1])
                        nc.vector.tensor_tensor(out=hTp[:, off + g0:off + g0 + gw],
                                                in0=xc[:, g0:g0 + gw],
                                                in1=sT[:, :gw], op=mybir.AluOpType.mult)
                        g0 += gw
                    off += w

            def ph1_flush(hTp, h_t, t_base, cn):
                """PE-transpose hTp[:, :cn] into node-major tiles and DMA to
                h_t[:, t_base:t_base+ceil(cn/128), :] (partition-major)."""
                nfull = cn // 128
                rem = cn - nfull * 128
                if nfull:
                    hst = hspool.tile([128, PIECE // 128, 128], bf16, tag="hst")
                    t0 = 0
                    while t0 < nfull:
                        tn = min(4, nfull - t0)
                        pt = pt_pool.tile([128, 512], bf16, tag="pt")
                        for q in range(tn):
                            nc.tensor.transpose(
                                pt[:, q * 128:(q + 1) * 128],
                                hTp[:, (t0 + q) * 128:(t0 + q + 1) * 128], ident[:])
                        nc.vector.tensor_copy(
                            out=hst[:, t0:t0 + tn, :].rearrange("p t d -> p (t d)"),
                            in_=pt[:, :tn * 128])
                        t0 += tn
                    nc.sync.dma_start(out=h_t[:, t_base:t_base + nfull, :],
                                      in_=hst[:, :nfull, :])
                if rem:
                    pt = pt_pool.tile([128, 512], bf16, tag="pt")
                    nc.tensor.transpose(pt[:rem, :128], hTp[:, nfull * 128:nfull * 128 + rem],
                                        ident[:])
                    tl = w1pool.tile([128, 128], bf16, tag="tail")
                    nc.vector.tensor_copy(out=tl[:rem, :], in_=pt[:rem, :128])
                    nc.sync.dma_start(
                        out=h_t[:rem, t_base + nfull:t_base + nfull + 1, :],
                        in_=tl[:rem, :].rearrange("p (t d) -> p t d", t=1))

            for s in range(NS):
                slo, shi, T = p.SEG[s]
                base = slo
                while base < shi:
                    cn = min(PIECE, shi - base)
                    hTp = htpool.tile([D, PIECE], bf16, tag="hT")
                    ph1_compute(hTp, base, cn)
                    ph1_flush(hTp, h_d[s], (base - slo) // 128, cn)
                    base += cn

            # ---- phase 2: NS scatter passes (stream s gathers from h_d[s])
            nq_counter = [0]
            g_tiles = [[None] * (Lg[s] // p.GCHUNK) for s in range(NS)]
            MGT = 4
            m_tiles = [[None] * (-(-S[s] // MGT)) for s in range(NS)]

            def ensure_chunk(s, ci):
                if g_tiles[s][ci] is not None:
                    return
                g = gpools[s].tile([128, p.GT, D], bf16, tag=f"g{s}")
                c0 = ci * (p.GCHUNK // 16)
                nc.gpsimd.dma_gather(
                    out_ap=g[:], in_ap=h_rows[s],
                    idxs_ap=idx_sb[s][:, c0:c0 + p.GCHUNK // 16],
                    num_idxs=p.GCHUNK, num_idxs_reg=p.GCHUNK, elem_size=D,
                    queue_num=nq_counter[0] % p.NQ)
                nq_counter[0] += 1
                g_tiles[s][ci] = g

            def ensure_mchunk(s, ci):
                """Build 4 one-hot tiles on-chip: M[e,d] = (lr[e] == d)."""
                if m_tiles[s][ci] is not None:
                    return
                t0 = ci * MGT
                tn = min(MGT, S[s] - t0)
                mt = mpool.tile([128, MGT, D], bf16, tag=f"m{s}")
                nc.vector.tensor_tensor(
                    out=mt[:, :tn, :],
                    in0=iota4[:, :tn * 128].rearrange("p (t d) -> p t d", d=128),
                    in1=lr_sb[s][:, t0:t0 + tn].broadcast_to([128, tn, 128]),
                    op=mybir.AluOpType.is_equal)
                m_tiles[s][ci] = mt

            pos = [np.concatenate([[0], np.cumsum(tS[s])]).astype(int)
                   for s in range(NS)]

            def scatter_group(s, b0, gn, pa):
                """Accumulate stream s's one-hot matmuls for blocks
                [b0, b0+gn) into PSUM pa."""
                for q in range(gn):
                    b = b0 + q
                    tcnt = int(tS[s][b])
                    for j in range(tcnt):
                        g = pos[s][b] + j
                        ensure_chunk(s, g // p.GT)
                        ensure_mchunk(s, g // MGT)
                        neigh = g_tiles[s][g // p.GT][:, g % p.GT, :]
                        M = m_tiles[s][g // MGT][:, g % MGT, :]
                        nc.tensor.matmul(pa[:, q * 128:(q + 1) * 128],
                                         lhsT=neigh, rhs=M,
                                         start=(j == 0), stop=(j == tcnt - 1))

            # passes 0..NS-2 buffered in SBUF
            aggs = []
            for s in range(NS - 1):
                cur = []
                b0 = 0
                while b0 < NBLK:
                    gn = min(4, NBLK - b0)
                    pa = pa_pool.tile([D, 512], f32, tag="pa")
                    scatter_group(s, b0, gn, pa)
                    agg = aggpools[s].tile([D, 512], bf16, tag=f"agg{s}")
                    nc.vector.tensor_copy(out=agg[:], in_=pa[:])
                    cur.append(agg)
                    b0 += gn
                aggs.append(cur)

            # ---- last pass + phase 3 fused per 4-block group
            sl_ = NS - 1
            OCH = 8
            ost = None
            ost_base = 0
            ost_n = 0
            b0 = 0
            while b0 < NBLK:
                gn = min(4, NBLK - b0)
                gi = b0 // 4
                pa = pa_pool.tile([D, 512], f32, tag="pa")
                scatter_group(sl_, b0, gn, pa)
                aggL = agg2pool.tile([D, 512], bf16, tag="aggL")
                nc.vector.tensor_copy(out=aggL[:], in_=pa[:])

                po = po_pool.tile([128, 512], f32, tag="po")
                for q in range(gn):
                    b = b0 + q
                    nb = min(128, NB - b * 128)
                    sl = slice(q * 128, q * 128 + D)
                    nc.tensor.matmul(po[:nb, sl],
                                     lhsT=xT_own[:, b * 128:b * 128 + nb],
                                     rhs=WnW[:], start=True, stop=False)
                    for s in range(NS - 1):
                        nc.tensor.matmul(po[:nb, sl],
                                         lhsT=aggs[s][gi][:, q * 128:q * 128 + nb],
                                         rhs=WgW[:], start=False, stop=False)
                    nc.tensor.matmul(po[:nb, sl],
                                     lhsT=aggL[:, q * 128:q * 128 + nb],
                                     rhs=WgW[:], start=False, stop=False)
                    nc.tensor.matmul(po[:nb, sl], lhsT=onesR[:1, :nb],
                                     rhs=biasR[:1, :], start=False, stop=True)
                for q in range(gn):
                    b = b0 + q
                    nb = min(128, NB - b * 128)
                    if ost is None:
                        ost = ospool.tile([128, OCH * D], f32, tag="ost")
                        ost_base = b
                        ost_n = 0
                    nc.scalar.activation(ost[:nb, ost_n * D:(ost_n + 1) * D],
                                         po[:nb, q * 128:q * 128 + D], AF.Tanh)
                    ost_n += 1
                    if (ost_n == OCH) or (b == NBLK - 1):
                        rows0 = ost_base * 128
                        nfull_o = ost_n if nb == 128 else ost_n - 1
                        if nfull_o:
                            nc.sync.dma_start(
                                out=out_d[rows0:rows0 + nfull_o * 128, :].rearrange(
                                    "(t p) d -> p t d", p=128),
                                in_=ost[:, :nfull_o * D].rearrange(
                                    "p (t d) -> p t d", d=D))
                        if nb != 128:
                            nc.sync.dma_start(
                                out=out_d[rows0 + nfull_o * 128:
                                          rows0 + nfull_o * 128 + nb, :],
                                in_=ost[:nb, nfull_o * D:nfull_o * D + D])
                        ost = None
                b0 += gn

    nc.compile()
    return nc


# ---------------------------------------------------------------- host entry

def _host_prep(p: P, x, edge_index, Wn_w, Wn_b, Wg_w, Wg_b, Wa_w, Wa_b):
    x = np.asarray(x, np.float32)
    xT = np.ascontiguousarray(x.T).astype(BF16)
    tS, L, Lg, per_core = prep_edges(
        p, np.asarray(edge_index)[0], np.asarray(edge_index)[1])

    shared = {
        "xT": xT,
        "WaW": np.asarray(Wa_w, np.float32).astype(BF16),
        "WaB": np.asarray(Wa_b, np.float32).reshape(p.D, 1),
        "WnW": np.asarray(Wn_w, np.float32).astype(BF16),
        "WgW": np.asarray(Wg_w, np.float32).astype(BF16),
        "biasR": (np.asarray(Wn_b, np.float32)
                  + np.asarray(Wg_b, np.float32)).reshape(1, p.D).astype(BF16),
        "onesR": np.ones((1, p.D), BF16),
        "ident": np.eye(p.D, dtype=np.float32).astype(BF16),
        "iota4": np.tile(np.arange(128, dtype=np.float32), 4)[None, :]
                 .repeat(128, 0).astype(BF16),
    }
    in_maps = []
    for k in range(p.NCORES):
        m = dict(shared)
        m["xT_own"] = np.ascontiguousarray(xT[:, k * p.NB:(k + 1) * p.NB])
        m.update(per_core[k])
        in_maps.append(m)
    return tS, L, Lg, in_maps


TRACE = False      # set True (e.g. from test.py) to capture an NTFF profile
LAST = None        # last BassKernelResults, for profiling/inspection


def kernel(**inputs) -> np.ndarray:
    global LAST
    from concourse import bass_utils
    bass_utils.upload_artifacts = lambda tmpdir: "local://" + tmpdir

    p = P()
    tS, L, Lg, in_maps = _host_prep(p, **inputs)
    nc = build(p, tS, L, Lg)
    kw = dict(trace=True, trace_cores=list(range(p.NCORES))) if TRACE else {}
    res = bass_utils.run_bass_kernel_spmd(
        nc, in_maps, core_ids=list(range(p.NCORES)), **kw)
    LAST = res
    out = np.concatenate([res.results[k]["out"] for k in range(p.NCORES)], axis=0)
    return out.astype(np.float32)
